# revision 1
# baseline (speedup 1.0000x reference)
"""Trainium2 Bass kernel for nn_Circuit_26654567039463.

kernel(**inputs) integrates dA/dt = i(omega + nu|A|^2)A + A @ T2t with
fixed-step dopri5 (99 intervals x 5 substeps) for a batch of 2048
trajectories, data-parallel over 8 NeuronCores (256 trajectories each).
See the formulation notes below; numerically validated against the jax
reference (relative error ~4.6e-4).
"""
import sys
for _p in ("/opt/trn_rl_repo",):
    if _p not in sys.path:
        sys.path.insert(0, _p)

import numpy as np

import concourse.bass as bass
import concourse.mybir as mybir
import concourse.tile as tile
from concourse import bacc
from concourse.tile import add_dep_helper

F32 = mybir.dt.float32
F32R = mybir.dt.float32r
BF16 = mybir.dt.bfloat16

MODES, INPUT_MODES, EVAL_PTS, T_END, SUBSTEPS = 64, 48, 100, 0.5, 5
N_INTERVALS_FULL = EVAL_PTS - 1
DT = T_END / (EVAL_PTS - 1)
H = DT / SUBSTEPS
B_CORE = 256  # batch per core

ATAB = {
    (2, 1): 0.2,
    (3, 1): 0.075, (3, 2): 0.225,
    (4, 1): 44 / 45, (4, 2): -56 / 15, (4, 3): 32 / 9,
    (5, 1): 19372 / 6561, (5, 2): -25360 / 2187, (5, 3): 64448 / 6561, (5, 4): -212 / 729,
    (6, 1): 9017 / 3168, (6, 2): -355 / 33, (6, 3): 46732 / 5247, (6, 4): 49 / 176,
    (6, 5): -5103 / 18656,
    (7, 1): 35 / 384, (7, 2): 0.0, (7, 3): 500 / 1113, (7, 4): 125 / 192,
    (7, 5): -2187 / 6784, (7, 6): 11 / 84,
}


# ---------------------------------------------------------------- host math
def make_T2(params, kappa, dtype=np.complex128):
    n = MODES
    M = np.concatenate([params, np.zeros((1,), params.dtype)]).reshape(n, n)
    Hh = 0.5 * (M + M.T)
    iH = (1j * Hh).astype(dtype)
    eye = np.eye(n, dtype=dtype)
    U = np.linalg.solve(eye + iH, eye - iH)
    UtU = U.T @ U
    mix = UtU @ np.linalg.inv(eye - UtU + np.array(1e-8, dtype) * eye)
    return -kappa[None, :].astype(dtype) * (0.5 * eye + mix)


def real_rep(M):
    """Real [128,128] G with (G @ S) == mode-major rep of a -> a @ M."""
    Mr, Mi = M.real, M.imag
    return np.block([[Mr.T, -Mi.T], [Mi.T, Mr.T]])


def build_weights(params, kappa, omega):
    """Returns (wmats [7,128,128] f32 as lhsT, index map)."""
    n = MODES
    T2 = make_T2(params.astype(np.float64), kappa.astype(np.float64))
    Wt = H * (T2.T + 1j * np.diag(omega.astype(np.float64)))
    L = real_rep(Wt)
    J = np.block([[np.zeros((n, n)), -np.eye(n)], [np.eye(n), np.zeros((n, n))]])
    I128 = np.eye(2 * n)

    # zeroth order + polynomial coefficients of P0_m in powers of L
    P0 = {1: I128}
    K0 = {}
    pc = {1: np.array([1., 0, 0, 0, 0, 0])}
    kc = {}
    for i in range(1, 7):
        Pi = I128.copy()
        pci = np.array([1., 0, 0, 0, 0, 0])
        for l in range(1, i):
            Pi = Pi + ATAB[(i, l)] * K0[l]
            pci = pci + ATAB[(i, l)] * kc[l]
        P0[i] = Pi
        pc[i] = pci
        K0[i] = L @ Pi
        kc[i] = np.roll(pci, 1); kc[i][0] = 0.0
    M0 = I128.copy()
    for i in range(1, 7):
        M0 = M0 + ATAB[(7, i)] * K0[i]
    # first order in g: hk_i correction = sum_{m<=i} E[i,m] W_m
    E = {}
    for i in range(1, 7):
        E[(i, i)] = J
        for m in range(1, i):
            acc = np.zeros((2 * n, 2 * n))
            for l in range(m, i):
                acc += ATAB[(i, l)] * (L @ E[(l, m)])
            E[(i, m)] = acc
    D = {}
    for m in range(1, 7):
        acc = np.zeros((2 * n, 2 * n))
        for i in range(m, 7):
            acc += ATAB[(7, i)] * E[(i, m)]
        D[m] = acc
    # regroup by powers of L: sum_m D_m (g.(P0_m y)) = sum_k F_k (g.(L^k y))
    F = {}
    for k in range(0, 2):
        acc = np.zeros((2 * n, 2 * n))
        for m in range(1, 7):
            acc += pc[m][k] * D[m]
        F[k] = acc

    # zeroth-order half-step map (dopri M0 with L/2) for midpoint gain eval
    def m0_of(Lx):
        P0h = {1: I128}
        K0h = {}
        for i in range(1, 7):
            Pi = I128.copy()
            for l in range(1, i):
                Pi = Pi + ATAB[(i, l)] * K0h[l]
            P0h[i] = Pi
            K0h[i] = Lx @ Pi
        Mh = I128.copy()
        for i in range(1, 7):
            Mh = Mh + ATAB[(7, i)] * K0h[i]
        return Mh
    M0h = m0_of(L * 0.5)
    VARIANT = "mid"
    if VARIANT == "mid":
        PG = M0h @ M0 @ M0 @ M0
        PH1 = M0h
        PH2 = M0h @ M0
    elif VARIANT == "pg_only":
        PG = M0h @ M0 @ M0
        PH1 = I128.copy()
        PH2 = M0
    else:
        PG = M0 @ M0
        PH1 = I128.copy()
        PH2 = M0

    mats = []
    idx = {}
    idx["A0"] = len(mats); mats.append((M0 - I128).T)   # fp32r
    idx["L"] = len(mats); mats.append(L.T)              # fp32r (setup only)
    idx["LM"] = len(mats); mats.append((L @ M0).T)      # fp32r
    idx["PG"] = len(mats); mats.append(PG.T)            # fp32r
    idx["F0"] = len(mats); mats.append(F[0].T)          # bf16
    idx["F1"] = len(mats); mats.append(F[1].T)          # bf16
    i64 = np.eye(n)
    idx["fold"] = len(mats); mats.append(np.block([[i64, i64], [i64, i64]]))  # bf16
    idx["PH1"] = len(mats); mats.append(PH1.T)          # fp32r (setup only)
    idx["PH2"] = len(mats); mats.append(PH2.T)          # fp32r (setup only)
    idx["PH3"] = len(mats); mats.append((M0h @ M0 @ M0).T)  # fp32r (setup only)
    wmats = np.stack(mats).astype(np.float32)
    return wmats, idx


def host_initial_state(A0_real, A0_imag, biases_real, biases_imag):
    """[128, B] mode-major initial padded state for a batch shard."""
    B = A0_real.shape[0]
    S = np.zeros((128, B), np.float32)
    S[:INPUT_MODES] = A0_real.T
    S[INPUT_MODES:MODES] = np.broadcast_to(biases_real[:, None], (MODES - INPUT_MODES, B))
    S[MODES:MODES + INPUT_MODES] = A0_imag.T
    S[MODES + INPUT_MODES:] = np.broadcast_to(biases_imag[:, None], (MODES - INPUT_MODES, B))
    return S


def host_scalevec(nonlinearity):
    s = np.sqrt(H * nonlinearity.astype(np.float64)).astype(np.float32)
    return np.concatenate([s, s]).reshape(128, 1)


# ---------------------------------------------------------------- kernel
def build_kernel(n_intervals, idx, n_dummy=0):
    NW = 10
    nc = bacc.Bacc("TRN2")
    s0_d = nc.dram_tensor("s0", [128, B_CORE], F32, kind="ExternalInput")
    w_d = nc.dram_tensor("wmats", [NW, 128, 128], F32, kind="ExternalInput")
    sc_d = nc.dram_tensor("scalevec", [128, 1], F32, kind="ExternalInput")
    traj_d = nc.dram_tensor("traj", [n_intervals, 128, B_CORE], F32,
                            kind="ExternalOutput")
    bf16_set = {idx["F0"], idx["F1"], idx["fold"]}

    with tile.TileContext(nc) as tc:
        import contextlib
        with contextlib.ExitStack() as ctx:
            singles = ctx.enter_context(tc.tile_pool(name="singles", bufs=1))
            wraw_p = ctx.enter_context(tc.tile_pool(name="wraw", bufs=2))
            state_p = ctx.enter_context(tc.tile_pool(name="state", bufs=3))
            work_p = ctx.enter_context(tc.tile_pool(name="work", bufs=3))
            z_psum = ctx.enter_context(tc.tile_pool(name="zpsum", bufs=2, space="PSUM"))
            d_psum = ctx.enter_context(tc.tile_pool(name="dpsum", bufs=2, space="PSUM"))
            g_psum = ctx.enter_context(tc.tile_pool(name="gpsum", bufs=4, space="PSUM"))

            # ---- one-time setup
            scv = singles.tile([128, 1], F32, tag="scv")
            nc.sync.dma_start(scv[:], sc_d[:])
            wts = []
            for i in range(NW):
                wraw = wraw_p.tile([128, 128], F32, tag="wraw")
                nc.sync.dma_start(wraw[:], w_d[i])
                wdt = BF16 if i in bf16_set else F32R
                wt = singles.tile([128, 128], wdt, tag=f"w{i}")
                nc.vector.tensor_copy(wt[:], wraw[:])
                wts.append(wt)

            y = state_p.tile([128, B_CORE], F32, tag="y")
            nc.sync.dma_start(y[:], s0_d[:])
            y_r = state_p.tile([128, B_CORE], F32R, tag="yr")
            nc.scalar.copy(y_r[:], y[:])

            # prefill gain pipeline for substeps 0 and 1
            gq = []
            for wname in ("PH1", "PH2", "PH3"):
                ue = z_psum.tile([128, B_CORE], F32, tag="zp")
                nc.tensor.matmul(ue[:], wts[idx[wname]][:], y_r[:],
                                 start=True, stop=True)
                sq = work_p.tile([128, B_CORE], BF16, tag="sq")
                nc.scalar.activation(sq[:], ue[:],
                                     mybir.ActivationFunctionType.Square,
                                     scale=scv[:])
                g2ps = g_psum.tile([128, B_CORE], F32, tag="g2ps")
                nc.tensor.matmul(g2ps[:], wts[idx["fold"]][:], sq[:],
                                 start=True, stop=True)
                g2 = work_p.tile([128, B_CORE], F32, tag="g2")
                nc.scalar.copy(g2[:], g2ps[:])
                gq.append((g2ps, g2))
            # prefill W1(0) = g(0) (.) (L y(0))
            z0 = z_psum.tile([128, B_CORE], F32, tag="zp")
            nc.tensor.matmul(z0[:], wts[idx["L"]][:], y_r[:], start=True, stop=True)
            W1 = work_p.tile([128, B_CORE], BF16, tag="W1")
            nc.vector.tensor_mul(W1[:], gq[0][1][:], z0[:])

            # PE warm-up: ~10us of back-to-back matmuls flips the HAM clock
            # gate to 8/8; steady-state PE gaps stay far below the ~5us idle
            # window that would re-throttle, so the whole run stays warm.
            junk = z_psum.tile([128, 128], F32, tag="zp")
            for _ in range(40):
                nc.tensor.matmul(junk[:], wts[idx["fold"]][:],
                                 wts[idx["F0"]][:], start=True, stop=True)

            pend_add = None  # deferred master update (y_old, delta)
            pend_fold = None  # deferred gain fold (sq_tile, prev_f1_mm)
            for interval in range(n_intervals):
                for sub in range(SUBSTEPS):
                    g2ps, g2 = gq.pop(0)
                    # W0 = g (.) y   [chain: gates F0]
                    W0 = work_p.tile([128, B_CORE], BF16, tag="W0")
                    nc.vector.tensor_mul(W0[:], g2ps[:], y_r[:])
                    if pend_add is not None:
                        y_old, dl_old = pend_add
                        y = state_p.tile([128, B_CORE], F32, tag="y")
                        nc.vector.tensor_add(y[:], y_old[:], dl_old[:])
                        pend_add = None
                    # delta group: A0 (start), F1 (input ready), F0 (stop, on-chain)
                    dl = d_psum.tile([128, B_CORE], F32, tag="dps")
                    nc.tensor.matmul(dl[:], wts[idx["A0"]][:], y_r[:],
                                     start=True, stop=False)
                    mm_f1 = nc.tensor.matmul(dl[:], wts[idx["F1"]][:], W1[:],
                                             start=False, stop=False)
                    if pend_fold is not None:
                        # previous substep's gain fold: input (sq) is ready by
                        # now, slotting it here keeps the PE stream gap-free
                        sq_o, f1_o = pend_fold
                        g2ps_n = g_psum.tile([128, B_CORE], F32, tag="g2ps")
                        mm_fold = nc.tensor.matmul(g2ps_n[:], wts[idx["fold"]][:],
                                                   sq_o[:], start=True, stop=True)
                        add_dep_helper(mm_fold.ins, f1_o.ins, sync=False,
                                       reason="fold after prev F1")
                        g2_n = work_p.tile([128, B_CORE], F32, tag="g2")
                        nc.scalar.copy(g2_n[:], g2ps_n[:])
                        gq.append((g2ps_n, g2_n))
                        pend_fold = None
                    mm_f0 = nc.tensor.matmul(dl[:], wts[idx["F0"]][:], W0[:],
                                             start=False, stop=True)
                    # prediction matmuls for next substep (inputs ready: y_r)
                    zp = z_psum.tile([128, B_CORE], F32, tag="zp")
                    mm_zp = nc.tensor.matmul(zp[:], wts[idx["LM"]][:], y_r[:],
                                             start=True, stop=True)
                    add_dep_helper(mm_zp.ins, mm_f0.ins, sync=False,
                                   reason="F0 before zpred in PE FIFO")
                    ue = z_psum.tile([128, B_CORE], F32, tag="zp")
                    mm_pg = nc.tensor.matmul(ue[:], wts[idx["PG"]][:], y_r[:],
                                             start=True, stop=True)
                    add_dep_helper(mm_pg.ins, mm_f0.ins, sync=False,
                                   reason="F0 before PG in PE FIFO")
                    # gain for substep n+2: square now, fold deferred into
                    # the next substep's PE stream
                    sq = work_p.tile([128, B_CORE], BF16, tag="sq")
                    nc.scalar.activation(sq[:], ue[:],
                                         mybir.ActivationFunctionType.Square,
                                         scale=scv[:])
                    pend_fold = (sq, mm_f1)
                    # state update: rounded copy first (it gates the next substep)
                    y_r2 = state_p.tile([128, B_CORE], F32R, tag="yr")
                    tt_yr = nc.vector.tensor_add(y_r2[:], y[:], dl[:])
                    # W1 for next substep, after the state update on DVE
                    W1 = work_p.tile([128, B_CORE], BF16, tag="W1")
                    tt_w1 = nc.vector.tensor_mul(W1[:], gq[0][1][:], zp[:])
                    add_dep_helper(tt_w1.ins, tt_yr.ins, sync=False,
                                   reason="state update before W1 on DVE")
                    if sub == SUBSTEPS - 1:
                        y_new = state_p.tile([128, B_CORE], F32, tag="y")
                        nc.vector.tensor_add(y_new[:], y[:], dl[:])
                        y = y_new
                    else:
                        pend_add = (y, dl)
                        y = None
                    y_r = y_r2
                nc.sync.dma_start(traj_d[interval], y[:])
    nc.compile()
    return nc


# ---------------------------------------------------------------- driver
_PROGRAM_CACHE = {}


def kernel(A0_real, A0_imag, params, biases_real, biases_imag,
           omega, kappa, nonlinearity):
    from concourse.bass_utils import run_bass_kernel_spmd

    NC_CORES = 8
    B = A0_real.shape[0]
    BS = B // NC_CORES
    assert BS == B_CORE, f"expected batch {NC_CORES * B_CORE}, got {B}"
    NI = N_INTERVALS_FULL

    wmats, idx = build_weights(np.asarray(params, np.float32),
                               np.asarray(kappa, np.float32),
                               np.asarray(omega, np.float32))
    scv = host_scalevec(np.asarray(nonlinearity, np.float32))

    key = NI
    if key not in _PROGRAM_CACHE:
        _PROGRAM_CACHE[key] = build_kernel(NI, idx)
    nc = _PROGRAM_CACHE[key]

    in_maps = []
    for c in range(NC_CORES):
        sl = slice(c * BS, (c + 1) * BS)
        S0 = host_initial_state(np.asarray(A0_real[sl], np.float32),
                                np.asarray(A0_imag[sl], np.float32),
                                np.asarray(biases_real, np.float32),
                                np.asarray(biases_imag, np.float32))
        in_maps.append({"s0": S0, "wmats": wmats, "scalevec": scv})

    res = run_bass_kernel_spmd(nc, in_maps, core_ids=list(range(NC_CORES)))

    out = np.empty((EVAL_PTS, B, MODES), np.complex64)
    for c in range(NC_CORES):
        sl = slice(c * BS, (c + 1) * BS)
        S0 = in_maps[c]["s0"]
        out[0, sl] = (S0[:MODES] + 1j * S0[MODES:]).T
        traj = res.results[c]["traj"]  # [NI, 128, BS] fp32
        out[1:, sl] = (traj[:, :MODES, :] + 1j * traj[:, MODES:, :]
                       ).transpose(0, 2, 1)
    return out



# revision 5
# speedup vs baseline: 3.8053x; 3.8053x over previous
"""Trainium2 Bass kernel for nn_Circuit_26654567039463.

Integrates dA/dt = i(omega + nu|A|^2)A + A @ T2^T for 2048 trajectories,
data-parallel over 8 NeuronCores (256 per core), with one fused step per
output interval (99 steps) instead of the reference's 5 dopri5 substeps:

  y_{n+1} = M5 y_n + B (g_n . z_n)          (real 128-dim rep, [128, B])

where M5 = R(L)^5 is the exact dopri5 5-substep linear propagator,
z_n ~ y(t_n + dt/2) is the mid-interval state predicted linearly from
y_{n-1} (z = M0^7.5 y_{n-1}), g_n = h*nu*|z_n|^2 is the nonlinear gain,
and B = 5i*M0^2.5 applies the first-order midpoint-quadrature correction
for the nonlinear phase over the interval.  Numerically validated against
the jax reference at rel err ~4.0e-3 (tolerance 2e-2).
"""
import sys
for _p in ("/opt/trn_rl_repo",):
    if _p not in sys.path:
        sys.path.insert(0, _p)

import numpy as np

import concourse.bass as bass
import concourse.mybir as mybir
import concourse.tile as tile
from concourse import bacc

F32 = mybir.dt.float32
F32R = mybir.dt.float32r
BF16 = mybir.dt.bfloat16

MODES, INPUT_MODES, EVAL_PTS, T_END, SUBSTEPS = 64, 48, 100, 0.5, 5
N_INTERVALS_FULL = EVAL_PTS - 1
DT = T_END / (EVAL_PTS - 1)
H = DT / SUBSTEPS
B_CORE = 256  # batch per core

ATAB = {
    (2, 1): 0.2,
    (3, 1): 0.075, (3, 2): 0.225,
    (4, 1): 44 / 45, (4, 2): -56 / 15, (4, 3): 32 / 9,
    (5, 1): 19372 / 6561, (5, 2): -25360 / 2187, (5, 3): 64448 / 6561, (5, 4): -212 / 729,
    (6, 1): 9017 / 3168, (6, 2): -355 / 33, (6, 3): 46732 / 5247, (6, 4): 49 / 176,
    (6, 5): -5103 / 18656,
    (7, 1): 35 / 384, (7, 2): 0.0, (7, 3): 500 / 1113, (7, 4): 125 / 192,
    (7, 5): -2187 / 6784, (7, 6): 11 / 84,
}


# ---------------------------------------------------------------- host math
def make_T2(params, kappa, dtype=np.complex128):
    n = MODES
    M = np.concatenate([params, np.zeros((1,), params.dtype)]).reshape(n, n)
    Hh = 0.5 * (M + M.T)
    iH = (1j * Hh).astype(dtype)
    eye = np.eye(n, dtype=dtype)
    U = np.linalg.solve(eye + iH, eye - iH)
    UtU = U.T @ U
    mix = UtU @ np.linalg.inv(eye - UtU + np.array(1e-8, dtype) * eye)
    return -kappa[None, :].astype(dtype) * (0.5 * eye + mix)


def dopri_linear_map(L):
    """Dopri5 one-substep map R(L) for dy/dtau = L y (tau in substep units)."""
    n = L.shape[0]
    I = np.eye(n, dtype=L.dtype)
    K = {}
    for i in range(1, 7):
        Pi = I.copy()
        for l in range(1, i):
            Pi = Pi + ATAB[(i, l)] * K[l]
        K[i] = L @ Pi
    M = I.copy()
    for i in range(1, 7):
        M = M + ATAB[(7, i)] * K[i]
    return M


def frac_power_series(M0, s, K=48):
    """M0^s via binomial series on X = M0 - I (converges, ||X|| < 1 here)."""
    X = M0 - np.eye(M0.shape[0], dtype=M0.dtype)
    out = np.eye(M0.shape[0], dtype=M0.dtype)
    term = np.eye(M0.shape[0], dtype=M0.dtype)
    c = 1.0
    for k in range(1, K + 1):
        c *= (s - (k - 1)) / k
        term = term @ X
        out = out + c * term
    return out


def rep(C):
    """Real rep of complex matrix C for column states S = [Re a; Im a]."""
    return np.block([[C.real, -C.imag], [C.imag, C.real]])


def build_weights(params, kappa, omega):
    """Returns (wmats [5,128,128] f32 stored as lhsT, index map)."""
    n = MODES
    T2 = make_T2(params.astype(np.float64), kappa.astype(np.float64))
    Lc = H * (T2 + 1j * np.diag(omega.astype(np.float64)))
    M0 = dopri_linear_map(Lc)
    M5 = np.linalg.matrix_power(M0, 5)
    Mh = frac_power_series(M0, 2.5)    # mid-interval predictor (lag 0)
    Z1 = frac_power_series(M0, 7.5)    # mid-interval predictor from y_{n-1}
    Bc = 5.0j * Mh                     # midpoint quadrature: int_0^5 ds -> w=5

    i64 = np.eye(n)
    mats, idx = [], {}
    idx["main"] = len(mats); mats.append(rep(M5 - np.eye(n)).T)  # f32r
    idx["zp"] = len(mats); mats.append(rep(Z1).T)                # f32r
    idx["zp0"] = len(mats); mats.append(rep(Mh).T)               # f32r
    idx["corr"] = len(mats); mats.append(rep(Bc).T)              # bf16
    idx["fold"] = len(mats); mats.append(np.block([[i64, i64], [i64, i64]]))
    wmats = np.stack(mats).astype(np.float32)
    return wmats, idx


def host_initial_state(A0_real, A0_imag, biases_real, biases_imag):
    """[128, B] mode-major initial padded state for a batch shard."""
    B = A0_real.shape[0]
    S = np.zeros((128, B), np.float32)
    S[:INPUT_MODES] = A0_real.T
    S[INPUT_MODES:MODES] = np.broadcast_to(biases_real[:, None], (MODES - INPUT_MODES, B))
    S[MODES:MODES + INPUT_MODES] = A0_imag.T
    S[MODES + INPUT_MODES:] = np.broadcast_to(biases_imag[:, None], (MODES - INPUT_MODES, B))
    return S


def host_scalevec(nonlinearity):
    s = np.sqrt(H * nonlinearity.astype(np.float64)).astype(np.float32)
    return np.concatenate([s, s]).reshape(128, 1)


# ---------------------------------------------------------------- kernel
def build_kernel(n_intervals, idx):
    NW = 5
    nc = bacc.Bacc("TRN2")
    s0_d = nc.dram_tensor("s0", [128, B_CORE], F32, kind="ExternalInput")
    w_d = nc.dram_tensor("wmats", [NW, 128, 128], F32, kind="ExternalInput")
    sc_d = nc.dram_tensor("scalevec", [128, 1], F32, kind="ExternalInput")
    traj_d = nc.dram_tensor("traj", [n_intervals, 128, B_CORE], F32R,
                            kind="ExternalOutput")
    bf16_set = {idx["corr"], idx["fold"]}

    with tile.TileContext(nc) as tc:
        import contextlib
        with contextlib.ExitStack() as ctx:
            singles = ctx.enter_context(tc.tile_pool(name="singles", bufs=1))
            wraw_p = ctx.enter_context(tc.tile_pool(name="wraw", bufs=2))
            state_p = ctx.enter_context(tc.tile_pool(name="state", bufs=3))
            work_p = ctx.enter_context(tc.tile_pool(name="work", bufs=3))
            z_psum = ctx.enter_context(tc.tile_pool(name="zpsum", bufs=2, space="PSUM"))
            d_psum = ctx.enter_context(tc.tile_pool(name="dpsum", bufs=2, space="PSUM"))
            g_psum = ctx.enter_context(tc.tile_pool(name="gpsum", bufs=2, space="PSUM"))

            # ---- one-time setup
            scv = singles.tile([128, 1], F32, tag="scv")
            nc.sync.dma_start(scv[:], sc_d[:])
            wts = []
            for i in range(NW):
                wraw = wraw_p.tile([128, 128], F32, tag="wraw")
                nc.sync.dma_start(wraw[:], w_d[i])
                wdt = BF16 if i in bf16_set else F32R
                wt = singles.tile([128, 128], wdt, tag=f"w{i}")
                nc.vector.tensor_copy(wt[:], wraw[:])
                wts.append(wt)

            y0 = state_p.tile([128, B_CORE], F32, tag="y")
            nc.sync.dma_start(y0[:], s0_d[:])
            y_r = state_p.tile([128, B_CORE], F32R, tag="yr")
            nc.scalar.copy(y_r[:], y0[:])

            # PE warm-up: ~10us of back-to-back matmuls flips the HAM clock
            # gate to 8/8; steady-state PE gaps stay far below the ~5us idle
            # window that would re-throttle, so the whole run stays warm.
            junk = z_psum.tile([128, 128], F32, tag="zp")
            for _ in range(40):
                nc.tensor.matmul(junk[:], wts[idx["fold"]][:],
                                 wts[idx["corr"]][:], start=True, stop=True)

            # ---- prefill: z/gain/u for step 0 via the lag-0 predictor
            z_ps = z_psum.tile([128, B_CORE], F32, tag="zp")
            nc.tensor.matmul(z_ps[:], wts[idx["zp0"]][:], y_r[:],
                             start=True, stop=True)
            sq = work_p.tile([128, B_CORE], BF16, tag="sq")
            nc.scalar.activation(sq[:], z_ps[:],
                                 mybir.ActivationFunctionType.Square,
                                 scale=scv[:])
            g_ps = g_psum.tile([128, B_CORE], F32, tag="gps")
            nc.tensor.matmul(g_ps[:], wts[idx["fold"]][:], sq[:],
                             start=True, stop=True)
            # DVE can only read one PSUM operand: stage z in SBUF (ACT)
            z_sb = work_p.tile([128, B_CORE], BF16, tag="zsb")
            nc.scalar.copy(z_sb[:], z_ps[:])
            u = work_p.tile([128, B_CORE], BF16, tag="u")
            nc.vector.tensor_mul(u[:], g_ps[:], z_sb[:])

            for n in range(n_intervals):
                last = (n == n_intervals - 1)
                # z_{n+1} first so the ACT square starts early
                if not last:
                    z_ps = z_psum.tile([128, B_CORE], F32, tag="zp")
                    nc.tensor.matmul(z_ps[:], wts[idx["zp"]][:], y_r[:],
                                     start=True, stop=True)
                # delta = (M5 - I) y_n + B u_n
                dl = d_psum.tile([128, B_CORE], F32, tag="dps")
                nc.tensor.matmul(dl[:], wts[idx["main"]][:], y_r[:],
                                 start=True, stop=False)
                nc.tensor.matmul(dl[:], wts[idx["corr"]][:], u[:],
                                 start=False, stop=True)
                if not last:
                    sq = work_p.tile([128, B_CORE], BF16, tag="sq")
                    nc.scalar.activation(sq[:], z_ps[:],
                                         mybir.ActivationFunctionType.Square,
                                         scale=scv[:])
                    g_ps = g_psum.tile([128, B_CORE], F32, tag="gps")
                    nc.tensor.matmul(g_ps[:], wts[idx["fold"]][:], sq[:],
                                     start=True, stop=True)
                    z_sb = work_p.tile([128, B_CORE], BF16, tag="zsb")
                    nc.scalar.copy(z_sb[:], z_ps[:])
                # state update (serial chain), then next step's u
                y_r2 = state_p.tile([128, B_CORE], F32R, tag="yr")
                nc.vector.tensor_add(y_r2[:], y_r[:], dl[:])
                if not last:
                    u = work_p.tile([128, B_CORE], BF16, tag="u")
                    nc.vector.tensor_mul(u[:], g_ps[:], z_sb[:])
                y_r = y_r2
                nc.sync.dma_start(traj_d[n], y_r[:])
    nc.compile()
    return nc


# ---------------------------------------------------------------- driver
_PROGRAM_CACHE = {}


def kernel(A0_real, A0_imag, params, biases_real, biases_imag,
           omega, kappa, nonlinearity):
    from concourse.bass_utils import run_bass_kernel_spmd

    NC_CORES = 8
    B = A0_real.shape[0]
    BS = B // NC_CORES
    assert BS == B_CORE, f"expected batch {NC_CORES * B_CORE}, got {B}"
    NI = N_INTERVALS_FULL

    wmats, idx = build_weights(np.asarray(params, np.float32),
                               np.asarray(kappa, np.float32),
                               np.asarray(omega, np.float32))
    scv = host_scalevec(np.asarray(nonlinearity, np.float32))

    key = NI
    if key not in _PROGRAM_CACHE:
        _PROGRAM_CACHE[key] = build_kernel(NI, idx)
    nc = _PROGRAM_CACHE[key]

    in_maps = []
    for c in range(NC_CORES):
        sl = slice(c * BS, (c + 1) * BS)
        S0 = host_initial_state(np.asarray(A0_real[sl], np.float32),
                                np.asarray(A0_imag[sl], np.float32),
                                np.asarray(biases_real, np.float32),
                                np.asarray(biases_imag, np.float32))
        in_maps.append({"s0": S0, "wmats": wmats, "scalevec": scv})

    res = run_bass_kernel_spmd(nc, in_maps, core_ids=list(range(NC_CORES)))

    out = np.empty((EVAL_PTS, B, MODES), np.complex64)
    for c in range(NC_CORES):
        sl = slice(c * BS, (c + 1) * BS)
        S0 = in_maps[c]["s0"]
        out[0, sl] = (S0[:MODES] + 1j * S0[MODES:]).T
        traj = res.results[c]["traj"]  # [NI, 128, BS] fp32
        out[1:, sl] = (traj[:, :MODES, :] + 1j * traj[:, MODES:, :]
                       ).transpose(0, 2, 1)
    return out
